# revision 33
# baseline (speedup 1.0000x reference)
"""Trainium2 Bass kernel for nn_ContextAttention (sparse_attention).

Math (per batch b):
  q = (x @ Wq + bq) / 16 ; k = x @ Wk + bk ; v0 = x @ Wv   (bv folded into bout)
  scoresT[t,s] = sum_d kT[d,t] qT[d,s]
  E1 = exp(scoresT); E1 *= exp(pe) on the 192-wide diagonal strip, in place
      (exp(pe) == 1 in bf16 beyond |t-s|<=2, so the strip covers pe exactly)
  E2 = E1' * band(|t-s|<=32)   (banded strips only)
  o1T[d,s] = sum_t V~[t,d] E1'[t,s] with V~=[V|1] -> row 64 = denominator d1
  o2T      = banded AV of the E2 strips (ones col gives band denominator)
  OT = o1T/d1 + o2T/d2   (x0.5 folded into Wout)
  out = OT.T @ (0.5*Wout) + (bv @ Wout + bout)

Sharding: data-parallel over batch across 8 cores (8 batches each). No
collectives.

v3 vs v2 (the 1.0 ms baseline):
  - pe correction merged INTO E1 in place -> o1 is a plain dense AV
    (removes the 10 correction matmuls per head).
  - normalization: one reciprocal_approx_fast per head on the merged
    [1,2,512] denominator rows (was 2x 3.3us iterative reciprocals =
    212us of the 1ms), one merged partition_broadcast, one merged norm
    multiply, one blend add.
  - o1/o2 live in one [128,2,512] psum tile per head.
  - V ones/zero columns in a persistent manually double-buffered const
    tile (no per-batch memsets).
"""

import sys

sys.path.insert(0, "/opt/trn_rl_repo")

import numpy as np

B, S, F, E, H, DH = 64, 512, 512, 256, 4, 64
HALF_WIN = 32
SCALE = 16.0  # EMBED ** 0.5
NCORES = 8
BPC = B // NCORES  # batches per core
TOK = BPC * S  # tokens per core


def _build():
    import concourse.bacc as bacc
    import concourse.tile as tile
    from concourse import mybir

    f32 = mybir.dt.float32
    f32r = mybir.dt.float32r
    bf16 = mybir.dt.bfloat16
    fp8 = mybir.dt.float8e4
    DR = mybir.MatmulPerfMode.DoubleRow
    # x is fp8; W{q,k} are fp8 pre-scaled by 32 (avoids fp8 subnormals).
    # scores psum = (32k)(32q) = 1024 * k.q ; softmax scale 1/16 folds in too.
    EXP_SCALE = 1.0 / (1024.0 * 16.0)
    Copy = mybir.ActivationFunctionType.Copy
    Exp = mybir.ActivationFunctionType.Exp
    mult = mybir.AluOpType.mult
    add = mybir.AluOpType.add

    nc = bacc.Bacc("TRN2", target_bir_lowering=False, debug=False)

    xT = nc.dram_tensor("xT", [F, TOK], fp8, kind="ExternalInput")
    wq_d = nc.dram_tensor("wq", [F, E], fp8, kind="ExternalInput")
    wk_d = nc.dram_tensor("wk", [F, E], fp8, kind="ExternalInput")
    wv_d = nc.dram_tensor("wv", [F, E], bf16, kind="ExternalInput")
    xTv = nc.dram_tensor("xTv", [F, TOK], bf16, kind="ExternalInput")
    wout_d = nc.dram_tensor("wout", [E, F], bf16, kind="ExternalInput")
    qkb_d = nc.dram_tensor("qkbias", [128, 4], f32, kind="ExternalInput")
    bout_d = nc.dram_tensor("boutr", [1, F], f32, kind="ExternalInput")
    estrip_d = nc.dram_tensor("estrip", [128, 192], bf16, kind="ExternalInput")
    band_d = nc.dram_tensor("bandmask", [128, 192], bf16, kind="ExternalInput")
    out_d = nc.dram_tensor("out", [TOK, F], f32, kind="ExternalOutput")

    with tile.TileContext(nc) as tc:
        with (
            tc.tile_pool(name="const", bufs=1) as const,
            tc.tile_pool(name="xt", bufs=2) as xpool,
            tc.tile_pool(name="qk", bufs=2) as qkpool,
            tc.tile_pool(name="ee", bufs=8) as epool,
            tc.tile_pool(name="st", bufs=8) as stpool,
            tc.tile_pool(name="rr", bufs=3) as rpool,
            tc.tile_pool(name="nn", bufs=3) as npool,
            tc.tile_pool(name="et", bufs=2) as etpool,
            tc.tile_pool(name="ot", bufs=2) as otpool,
            tc.tile_pool(name="ff", bufs=2) as fpool,
            tc.tile_pool(name="ps", bufs=2, space="PSUM") as pspool,
            tc.tile_pool(name="sc", bufs=1, space="PSUM") as scpool,
            tc.tile_pool(name="po", bufs=4, space="PSUM") as popool,
        ):
            # ---- persistent constants (spread across engine queues so the
            # critical path to the first matmul is short) ----
            wq_sb = const.tile([128, 4, E], fp8, tag="wq")
            nc.sync.dma_start(
                wq_sb[:], wq_d.rearrange("(c p) e -> p c e", p=128)
            )
            wk_sb = const.tile([128, 4, E], fp8, tag="wk")
            nc.scalar.dma_start(
                wk_sb[:], wk_d.rearrange("(c p) e -> p c e", p=128)
            )
            wv_sb = const.tile([128, 4, E], bf16, tag="wv")
            nc.gpsimd.dma_start(
                wv_sb[:], wv_d.rearrange("(c p) e -> p c e", p=128)
            )
            wout_sb = const.tile([128, 2, F], bf16, tag="wout")
            nc.gpsimd.dma_start(
                wout_sb[:], wout_d.rearrange("(c p) e -> p c e", p=128)
            )
            estrip_sb = const.tile([128, 192], bf16, tag="estrip")
            nc.gpsimd.dma_start(estrip_sb[:], estrip_d[:, :])
            qkb_sb = const.tile([128, 4], f32, tag="qkb")
            nc.scalar.dma_start(qkb_sb[:], qkb_d[:, :])
            band_sb = const.tile([128, 192], bf16, tag="band")
            nc.scalar.dma_start(band_sb[:], band_d[:, :])
            bout_row = const.tile([1, F], f32, tag="boutrow")
            nc.gpsimd.dma_start(bout_row[:], bout_d[0:1, :])
            bout_b = const.tile([128, F], f32, tag="boutb")
            nc.gpsimd.partition_broadcast(bout_b[:], bout_row[:])
            # V tiles: [128t, slot, ttile, head, 128] with col 64 = ones
            # (denominator) and cols 65:128 = 0 (keeps M=128 so FWL stays
            # on). Ones/zeros written ONCE; per-batch V-copies only touch
            # cols 0:64.
            vt_all = const.tile([128, 2, 4, 4, 65], bf16, tag="vt")
            nc.gpsimd.memset(vt_all[:, :, :, :, 64:65], 1.0)

            def load_xt(b):
                xt = xpool.tile([128, 4, S], fp8, tag="xt")
                nc.sync.dma_start(
                    xt[:],
                    xT.rearrange("(c p) t -> p c t", p=128)[
                        :, :, 512 * b : 512 * (b + 1)
                    ],
                )
                xtv = xpool.tile([128, 4, S], bf16, tag="xtv")
                nc.sync.dma_start(
                    xtv[:],
                    xTv.rearrange("(c p) t -> p c t", p=128)[
                        :, :, 512 * b : 512 * (b + 1)
                    ],
                )
                return xt, xtv

            def make_qkv_thunks(xt, xtv, b):
                """Q/K/V projections for batch b as 8 weaveable thunks.
                Output tiles are allocated eagerly so callers can reference
                them before the thunks have emitted."""
                QP, KP = [], []
                slot = b % 2
                thunks = []
                for et in range(2):
                    for lst, w_sb, bcol in ((QP, wq_sb, 0), (KP, wk_sb, 2)):
                        t = qkpool.tile(
                            [128, S], bf16, name=f"qk{b}_{bcol}_{et}",
                            tag=f"{'q' if bcol == 0 else 'k'}p{et}",
                        )
                        lst.append(t)

                        def th(et=et, w_sb=w_sb, bcol=bcol, t=t):
                            ps = pspool.tile(
                                [128, S], f32, name=f"ps{b}_{bcol}_{et}",
                                tag="ps",
                            )
                            for kc in range(2):
                                nc.tensor.matmul(
                                    ps[:],
                                    w_sb[
                                        :, 2 * kc : 2 * kc + 2,
                                        128 * et : 128 * (et + 1),
                                    ],
                                    xt[:, 2 * kc : 2 * kc + 2, :],
                                    start=(kc == 0),
                                    stop=(kc == 1),
                                    perf_mode=DR,
                                )
                            nc.scalar.add(
                                t[:], ps[:],
                                qkb_sb[:, bcol + et : bcol + et + 1],
                            )

                        thunks.append(th)
                for j in range(4):

                    def th(j=j):
                        ps = pspool.tile(
                            [128, E], f32, name=f"psv{b}_{j}", tag="ps"
                        )
                        for kc in range(4):
                            nc.tensor.matmul(
                                ps[:],
                                xtv[:, kc, 128 * j : 128 * (j + 1)],
                                wv_sb[:, kc, :],
                                start=(kc == 0),
                                stop=(kc == 3),
                            )
                        nc.vector.tensor_copy(
                            vt_all[:, slot, j, :, 0:64],
                            ps.rearrange("p (h x) -> p h x", x=64),
                        )

                    thunks.append(th)
                return thunks, (QP, KP, slot)

            def qkv_proj(xt, xtv, b):
                thunks, ctx = make_qkv_thunks(xt, xtv, b)
                for th in thunks:
                    th()
                return ctx

            xt, xtv = load_xt(0)
            QP, KP, vslot = qkv_proj(xt, xtv, 0)

            def make_outproj_thunks(OT, b):
                fs = fpool.tile([128, 4, F], f32, name=f"fs{b}", tag="fs")
                thunks = []
                for j in range(4):

                    def th(j=j):
                        fp = pspool.tile(
                            [128, F], f32, name=f"fp{b}_{j}", tag="ps"
                        )
                        nc.tensor.matmul(
                            fp[:],
                            OT[0][:, 128 * j : 128 * (j + 1)],
                            wout_sb[:, 0, :],
                            start=True,
                            stop=False,
                        )
                        nc.tensor.matmul(
                            fp[:],
                            OT[1][:, 128 * j : 128 * (j + 1)],
                            wout_sb[:, 1, :],
                            start=False,
                            stop=True,
                        )
                        nc.vector.tensor_tensor(
                            fs[:, j, :], fp[:], bout_b[:], add
                        )
                        if j == 1 or j == 3:
                            nc.sync.dma_start(
                                out_d.rearrange(
                                    "(bb j p) f -> p (bb j) f", p=128, j=4
                                )[:, 4 * b + j - 1 : 4 * b + j + 1, :],
                                fs[:, j - 1 : j + 1, :],
                            )

                    thunks.append(th)
                return thunks

            def do_outproj(OT, b):
                for th in make_outproj_thunks(OT, b):
                    th()

            class Front:
                """scores + exp + strips for one head, emission split into
                weaveable pieces: mm(tt) emits one scores matmul; done(pp)
                emits the exp (and for pp=1 the strip multiplies)."""

                def __init__(self, h, QP, KP, gtag):
                    self.h, self.QP, self.KP = h, QP, KP
                    self.gtag = gtag
                    self.sp = {}
                    self.E1s = []
                    self.E2s = []

                def mm(self, tt):
                    h, et, hl = self.h, self.h // 2, self.h % 2
                    pp = tt // 2
                    if tt % 2 == 0:
                        self.sp[pp] = scpool.tile(
                            [128, 2, S], f32, name=f"sc{self.gtag}_{pp}",
                            tag="sc",
                        )
                    nc.tensor.matmul(
                        self.sp[pp][:, tt % 2, :],
                        self.KP[et][
                            64 * hl : 64 * hl + 64, 128 * tt : 128 * (tt + 1)
                        ],
                        self.QP[et][64 * hl : 64 * hl + 64, :],
                        start=True,
                        stop=True,
                        skip_group_check=True,
                    )

                def done(self, pp):
                    e1 = epool.tile(
                        [128, 2, S], bf16, name=f"e1{self.gtag}_{pp}", tag="e1"
                    )
                    nc.scalar.activation(
                        e1[:], self.sp[pp][:], Exp, scale=EXP_SCALE
                    )
                    self.E1s.append(e1[:, 0, :])
                    self.E1s.append(e1[:, 1, :])
                    if pp == 0:
                        return
                    # strips: in-place pe merge (E1 -> E1', DVE) and banded
                    # E2 = E1' * band (Pool). Strip tt covers s in
                    # [128tt-32, 128tt+160); local l in [lo, hi) clipped.
                    for tt in range(4):
                        lo = 32 if tt == 0 else 0
                        hi = 160 if tt == 3 else 192
                        reg = self.E1s[tt][
                            :, 128 * tt - 32 + lo : 128 * tt - 32 + hi
                        ]
                        nc.vector.tensor_tensor(
                            reg, reg, estrip_sb[:, lo:hi], mult
                        )
                        st = stpool.tile(
                            [128, 192], bf16, name=f"e2{self.gtag}_{tt}",
                            tag="e2",
                        )
                        nc.gpsimd.tensor_tensor(
                            st[:, lo:hi], reg, band_sb[:, lo:hi], mult
                        )
                        self.E2s.append(st)

                def run_all(self):
                    for tt in range(4):
                        self.mm(tt)
                        if tt % 2 == 1:
                            self.done(tt // 2)

            pending = []  # weaveable big-MM thunks (qkv / outproj chunks)

            def head_back(h, vslot, ET, E1s, E2s, nxt=None):
                """transposed AV + wide normalization + blend for head h."""
                et, hl = h // 2, h % 2
                # ---- transposed AV: per s-chunk st, out [128s, 65] =
                # E1'^T @ V~ (V~ = [V | ones] moving, N=65). Col 64 is the
                # per-s denominator -> wide per-partition reciprocal. ----
                PT1 = popool.tile([128, 4, 128], f32, tag="po")
                PT2 = popool.tile([128, 4, 128], f32, tag="po")
                vaug = vt_all[:, vslot]
                for st in range(4):
                    # weave big-stream matmuls (qkv/outproj chunks and the
                    # next head's scores) between the small AVT clusters to
                    # keep the PE array dense
                    if pending:
                        pending.pop(0)()
                    if nxt is not None:
                        nxt.mm(st)
                        if st % 2 == 1:
                            nxt.done(st // 2)
                    for tt in range(4):
                        nc.tensor.matmul(
                            PT1[:, st, 0:65],
                            E1s[tt][:, 128 * st : 128 * (st + 1)],
                            vaug[:, tt, h, :],
                            start=(tt == 0),
                            stop=(tt == 3),
                            skip_group_check=True,
                        )
                    # band: main strip tt=st covers the whole chunk; strip
                    # tt=st-1 covers s-subrange [0:32), tt=st+1 [96:128).
                    nc.tensor.matmul(
                        PT2[:, st, 0:65],
                        E2s[st][:, 32:160],
                        vaug[:, st, h, :],
                        start=True,
                        stop=False,
                        skip_group_check=True,
                    )
                    if st > 0:
                        nc.tensor.matmul(
                            PT2[0:32, st, 0:65],
                            E2s[st - 1][:, 160:192],
                            vaug[:, st - 1, h, :],
                            start=False,
                            stop=(st == 3),
                            skip_group_check=True,
                        )
                    if st < 3:
                        nc.tensor.matmul(
                            PT2[96:128, st, 0:65],
                            E2s[st + 1][:, 0:32],
                            vaug[:, st + 1, h, :],
                            start=False,
                            stop=True,
                            skip_group_check=True,
                            tile_position=(0, 96),
                        )

                # ---- normalization, all wide: [128,4] reciprocals,
                # free-dim-broadcast multiplies, blend into the ET tile ----
                rT1 = rpool.tile([128, 4], f32, tag="rt1")
                nc.vector.reciprocal(rT1[:], PT1[:, :, 64:65])
                rT2 = rpool.tile([128, 4], f32, tag="rt2")
                nc.vector.reciprocal(rT2[:], PT2[:, :, 64:65])
                tT1 = npool.tile([128, 4, 64], bf16, tag="t1")
                nc.vector.tensor_tensor(
                    tT1[:], PT1[:, :, 0:64],
                    rT1[:, :, None].broadcast_to((128, 4, 64)), mult,
                )
                tT2 = npool.tile([128, 4, 64], bf16, tag="t2")
                nc.vector.tensor_tensor(
                    tT2[:], PT2[:, :, 0:64],
                    rT2[:, :, None].broadcast_to((128, 4, 64)), mult,
                )
                nc.gpsimd.tensor_tensor(ET[et][:, :, hl, :], tT1[:], tT2[:], add)

            # ---- software-pipelined head stream: emit scores/exp/strips
            # for head g+1 BEFORE the AV/normalize of head g, so the PE
            # chews AV(g) while ACT/DVE/Pool produce head g+1's strips ----
            NG = BPC * H
            bctx = {0: (QP, KP, vslot)}  # per-batch (QP, KP, vslot)
            ET_all = {}
            OT_all = {}
            xt_next = None
            fronts = {}
            fronts[0] = Front(0, QP, KP, "g0")
            fronts[0].run_all()
            for g in range(NG):
                b, h = g // H, g % H
                if h == 0:
                    ET_all[b] = [
                        etpool.tile(
                            [128, 4, 2, 64], bf16, name=f"et{c}_{b}",
                            tag=f"et{c}",
                        )
                        for c in range(2)
                    ]
                    OT_all[b] = [None, None]
                    if b + 1 < BPC:
                        xt_next = load_xt(b + 1)
                nxt = None
                if g + 1 < NG:
                    QPf, KPf, _ = bctx[(g + 1) // H]
                    nxt = Front((g + 1) % H, QPf, KPf, f"g{g + 1}")
                    fronts[g + 1] = nxt
                fr = fronts.pop(g)
                _, _, vs = bctx[b]
                head_back(h, vs, ET_all[b], fr.E1s, fr.E2s, nxt=nxt)
                if h % 2 == 1:
                    # assemble OT[et] = ET[et].T via the DMA XBAR
                    et = h // 2
                    ot = otpool.tile(
                        [128, S], bf16, name=f"ot{et}_{b}", tag=f"ot{et}"
                    )
                    for st in range(4):
                        nc.sync.dma_start(
                            ot[:, 128 * st : 128 * (st + 1)],
                            ET_all[b][et][:, st, :, :],
                            transpose=True,
                        )
                    OT_all[b][et] = ot
                if h == 0 and b > 0:
                    # deferred out-proj of the previous batch (woven)
                    pending.extend(make_outproj_thunks(OT_all[b - 1], b - 1))
                if h == 1 and b + 1 < BPC:
                    # next batch's projections early (woven)
                    thunks, ctx = make_qkv_thunks(*xt_next, b + 1)
                    bctx[b + 1] = ctx
                    pending.extend(thunks)
            for th in pending:
                th()
            do_outproj(OT_all[BPC - 1], BPC - 1)

    nc.compile()
    return nc


_CACHE = {}
LAST_RESULTS = None


def prep_in_maps(inputs, Wq, bq, Wk, bk, Wv, bv, gamma, theta, Wout, bout):
    import ml_dtypes

    bfloat16 = ml_dtypes.bfloat16

    x = np.asarray(inputs, np.float32)
    Wq = np.asarray(Wq, np.float32)
    bq = np.asarray(bq, np.float32)
    Wk = np.asarray(Wk, np.float32)
    bk = np.asarray(bk, np.float32)
    Wv = np.asarray(Wv, np.float32)
    bv = np.asarray(bv, np.float32)
    Wout = np.asarray(Wout, np.float32)
    bout = np.asarray(bout, np.float32)
    gamma = float(np.asarray(gamma))
    theta = float(np.asarray(theta))

    # host-side prep. W{q,k} scaled by 32 for fp8 range; the projection
    # outputs are then 32x, scores 1024x -> compensated in EXP_SCALE
    # (with the softmax 1/sqrt(E)).
    WSC = 32.0
    fp8 = ml_dtypes.float8_e4m3
    wq_8 = (WSC * Wq).astype(fp8)
    wk_8 = (WSC * Wk).astype(fp8)
    wv_b = Wv.astype(bfloat16)
    qkb = (WSC * np.stack(
        [bq[:128], bq[128:], bk[:128], bk[128:]], axis=1
    )).astype(np.float32)  # [128, 4]
    bout_p = (bout + bv @ Wout).astype(np.float32).reshape(1, F)
    wout_h = (0.5 * Wout).astype(bfloat16)
    # strip coords: l = s - (128tt - 32); delta = t - s = p - l + 32.
    # estrip = exp(pe(delta)) (== 1 in bf16 beyond |delta|<=2);
    # bandmask = 1 where |delta| <= HALF_WIN else 0.
    p_i = np.arange(128)[:, None]
    l_i = np.arange(192)[None, :]
    delta = (p_i - l_i + 32).astype(np.float32)
    pe_val = np.exp(-np.abs(gamma * delta * delta - theta)).astype(np.float32)
    band = (np.abs(delta) <= HALF_WIN).astype(np.float32)
    estrip = np.exp(pe_val).astype(bfloat16)
    bandmask = band.astype(bfloat16)

    shared = {
        "wq": np.ascontiguousarray(wq_8),
        "wk": np.ascontiguousarray(wk_8),
        "wv": np.ascontiguousarray(wv_b),
        "wout": np.ascontiguousarray(wout_h),
        "qkbias": np.ascontiguousarray(qkb),
        "boutr": bout_p,
        "estrip": np.ascontiguousarray(estrip),
        "bandmask": np.ascontiguousarray(bandmask),
    }
    in_maps = []
    for c in range(NCORES):
        xc = x[c * BPC : (c + 1) * BPC].reshape(TOK, F)
        m = dict(shared)
        xct = xc.T
        m["xT"] = np.ascontiguousarray(xct.astype(fp8))
        m["xTv"] = np.ascontiguousarray(xct.astype(bfloat16))
        in_maps.append(m)
    return in_maps


def get_nc():
    if "nc" not in _CACHE:
        _CACHE["nc"] = _build()
    return _CACHE["nc"]


def kernel(inputs, Wq, bq, Wk, bk, Wv, bv, gamma, theta, Wout, bout):
    global LAST_RESULTS
    from concourse.bass_utils import run_bass_kernel_spmd

    in_maps = prep_in_maps(
        inputs, Wq, bq, Wk, bk, Wv, bv, gamma, theta, Wout, bout
    )
    nc = get_nc()
    res = run_bass_kernel_spmd(nc, in_maps, core_ids=list(range(NCORES)))
    LAST_RESULTS = res
    out = np.concatenate(
        [res.results[c]["out"].reshape(BPC, S, F) for c in range(NCORES)], axis=0
    )
    return out


# revision 34
# speedup vs baseline: 1.3312x; 1.3312x over previous
"""Trainium2 Bass kernel for nn_ContextAttention (sparse_attention).

Math (per batch b):
  q = (x @ Wq + bq) / 16 ; k = x @ Wk + bk ; v0 = x @ Wv   (bv folded into bout)
  scoresT[t,s] = sum_d kT[d,t] qT[d,s]
  E1 = exp(scoresT); E1 *= exp(pe) on the 192-wide diagonal strip, in place
      (exp(pe) == 1 in bf16 beyond |t-s|<=2, so the strip covers pe exactly)
  E2 = E1' * band(|t-s|<=32)   (banded strips only)
  o1T[d,s] = sum_t V~[t,d] E1'[t,s] with V~=[V|1] -> row 64 = denominator d1
  o2T      = banded AV of the E2 strips (ones col gives band denominator)
  OT = o1T/d1 + o2T/d2   (x0.5 folded into Wout)
  out = OT.T @ (0.5*Wout) + (bv @ Wout + bout)

Sharding: data-parallel over batch across 8 cores (8 batches each). No
collectives.

v3 vs v2 (the 1.0 ms baseline):
  - pe correction merged INTO E1 in place -> o1 is a plain dense AV
    (removes the 10 correction matmuls per head).
  - normalization: one reciprocal_approx_fast per head on the merged
    [1,2,512] denominator rows (was 2x 3.3us iterative reciprocals =
    212us of the 1ms), one merged partition_broadcast, one merged norm
    multiply, one blend add.
  - o1/o2 live in one [128,2,512] psum tile per head.
  - V ones/zero columns in a persistent manually double-buffered const
    tile (no per-batch memsets).
"""

import sys

sys.path.insert(0, "/opt/trn_rl_repo")

import numpy as np

B, S, F, E, H, DH = 64, 512, 512, 256, 4, 64
HALF_WIN = 32
SCALE = 16.0  # EMBED ** 0.5
NCORES = 8
BPC = B // NCORES  # batches per core
TOK = BPC * S  # tokens per core


def _build():
    import concourse.bacc as bacc
    import concourse.tile as tile
    from concourse import mybir

    f32 = mybir.dt.float32
    f32r = mybir.dt.float32r
    bf16 = mybir.dt.bfloat16
    fp8 = mybir.dt.float8e4
    DR = mybir.MatmulPerfMode.DoubleRow
    # x is fp8; W{q,k} are fp8 pre-scaled by 32 (avoids fp8 subnormals).
    # scores psum = (32k)(32q) = 1024 * k.q ; softmax scale 1/16 folds in too.
    EXP_SCALE = 1.0 / (1024.0 * 16.0)
    Copy = mybir.ActivationFunctionType.Copy
    Exp = mybir.ActivationFunctionType.Exp
    mult = mybir.AluOpType.mult
    add = mybir.AluOpType.add

    nc = bacc.Bacc("TRN2", target_bir_lowering=False, debug=False)

    xT = nc.dram_tensor("xT", [F, TOK], fp8, kind="ExternalInput")
    wq_d = nc.dram_tensor("wq", [F, E], fp8, kind="ExternalInput")
    wk_d = nc.dram_tensor("wk", [F, E], fp8, kind="ExternalInput")
    wv_d = nc.dram_tensor("wv", [F, E], bf16, kind="ExternalInput")
    xTv = nc.dram_tensor("xTv", [F, TOK], bf16, kind="ExternalInput")
    wout_d = nc.dram_tensor("wout", [E, F], bf16, kind="ExternalInput")
    qkb_d = nc.dram_tensor("qkbias", [128, 4], f32, kind="ExternalInput")
    bout_d = nc.dram_tensor("boutr", [1, F], f32, kind="ExternalInput")
    estrip_d = nc.dram_tensor("estrip", [128, 192], bf16, kind="ExternalInput")
    band_d = nc.dram_tensor("bandmask", [128, 192], bf16, kind="ExternalInput")
    out_d = nc.dram_tensor("out", [TOK, F], f32, kind="ExternalOutput")

    with tile.TileContext(nc) as tc:
        with (
            tc.tile_pool(name="const", bufs=1) as const,
            tc.tile_pool(name="xt", bufs=2) as xpool,
            tc.tile_pool(name="qk", bufs=2) as qkpool,
            tc.tile_pool(name="ee", bufs=8) as epool,
            tc.tile_pool(name="st", bufs=8) as stpool,
            tc.tile_pool(name="rr", bufs=3) as rpool,
            tc.tile_pool(name="nn", bufs=3) as npool,
            tc.tile_pool(name="et", bufs=2) as etpool,
            tc.tile_pool(name="ot", bufs=2) as otpool,
            tc.tile_pool(name="ff", bufs=2) as fpool,
            tc.tile_pool(name="ps", bufs=2, space="PSUM") as pspool,
            tc.tile_pool(name="sc", bufs=1, space="PSUM") as scpool,
            tc.tile_pool(name="po", bufs=4, space="PSUM") as popool,
        ):
            # ---- persistent constants (spread across engine queues so the
            # critical path to the first matmul is short) ----
            wq_sb = const.tile([128, 4, E], fp8, tag="wq")
            nc.sync.dma_start(
                wq_sb[:], wq_d.rearrange("(c p) e -> p c e", p=128)
            )
            wk_sb = const.tile([128, 4, E], fp8, tag="wk")
            nc.scalar.dma_start(
                wk_sb[:], wk_d.rearrange("(c p) e -> p c e", p=128)
            )
            wv_sb = const.tile([128, 4, E], bf16, tag="wv")
            nc.gpsimd.dma_start(
                wv_sb[:], wv_d.rearrange("(c p) e -> p c e", p=128)
            )
            wout_sb = const.tile([128, 2, F], bf16, tag="wout")
            nc.gpsimd.dma_start(
                wout_sb[:], wout_d.rearrange("(c p) e -> p c e", p=128)
            )
            estrip_sb = const.tile([128, 192], bf16, tag="estrip")
            nc.gpsimd.dma_start(estrip_sb[:], estrip_d[:, :])
            qkb_sb = const.tile([128, 4], f32, tag="qkb")
            nc.scalar.dma_start(qkb_sb[:], qkb_d[:, :])
            band_sb = const.tile([128, 192], bf16, tag="band")
            nc.scalar.dma_start(band_sb[:], band_d[:, :])
            bout_row = const.tile([1, F], f32, tag="boutrow")
            nc.gpsimd.dma_start(bout_row[:], bout_d[0:1, :])
            bout_b = const.tile([128, F], f32, tag="boutb")
            nc.gpsimd.partition_broadcast(bout_b[:], bout_row[:])
            # V tiles: [128t, slot, ttile, head, 128] with col 64 = ones
            # (denominator) and cols 65:128 = 0 (keeps M=128 so FWL stays
            # on). Ones/zeros written ONCE; per-batch V-copies only touch
            # cols 0:64.
            vt_all = const.tile([128, 2, 4, 4, 65], bf16, tag="vt")
            nc.gpsimd.memset(vt_all[:, :, :, :, 64:65], 1.0)

            def load_xt(b):
                xt = xpool.tile([128, 4, S], fp8, tag="xt")
                nc.sync.dma_start(
                    xt[:],
                    xT.rearrange("(c p) t -> p c t", p=128)[
                        :, :, 512 * b : 512 * (b + 1)
                    ],
                )
                xtv = xpool.tile([128, 4, S], bf16, tag="xtv")
                nc.sync.dma_start(
                    xtv[:],
                    xTv.rearrange("(c p) t -> p c t", p=128)[
                        :, :, 512 * b : 512 * (b + 1)
                    ],
                )
                return xt, xtv

            def make_qkv_thunks(xt, xtv, b):
                """Q/K/V projections for batch b as 8 weaveable thunks.
                Output tiles are allocated eagerly so callers can reference
                them before the thunks have emitted."""
                QP, KP = [], []
                slot = b % 2
                thunks = []
                for et in range(2):
                    for lst, w_sb, bcol in ((QP, wq_sb, 0), (KP, wk_sb, 2)):
                        t = qkpool.tile(
                            [128, S], bf16, name=f"qk{b}_{bcol}_{et}",
                            tag=f"{'q' if bcol == 0 else 'k'}p{et}",
                        )
                        lst.append(t)

                        def th(et=et, w_sb=w_sb, bcol=bcol, t=t):
                            ps = pspool.tile(
                                [128, S], f32, name=f"ps{b}_{bcol}_{et}",
                                tag="ps",
                            )
                            for kc in range(2):
                                nc.tensor.matmul(
                                    ps[:],
                                    w_sb[
                                        :, 2 * kc : 2 * kc + 2,
                                        128 * et : 128 * (et + 1),
                                    ],
                                    xt[:, 2 * kc : 2 * kc + 2, :],
                                    start=(kc == 0),
                                    stop=(kc == 1),
                                    perf_mode=DR,
                                )
                            nc.scalar.add(
                                t[:], ps[:],
                                qkb_sb[:, bcol + et : bcol + et + 1],
                            )

                        thunks.append(th)
                for j in range(4):

                    def th(j=j):
                        ps = pspool.tile(
                            [128, E], f32, name=f"psv{b}_{j}", tag="ps"
                        )
                        for kc in range(4):
                            nc.tensor.matmul(
                                ps[:],
                                xtv[:, kc, 128 * j : 128 * (j + 1)],
                                wv_sb[:, kc, :],
                                start=(kc == 0),
                                stop=(kc == 3),
                            )
                        nc.vector.tensor_copy(
                            vt_all[:, slot, j, :, 0:64],
                            ps.rearrange("p (h x) -> p h x", x=64),
                        )

                    thunks.append(th)
                return thunks, (QP, KP, slot)

            def qkv_proj(xt, xtv, b):
                thunks, ctx = make_qkv_thunks(xt, xtv, b)
                for th in thunks:
                    th()
                return ctx

            xt, xtv = load_xt(0)
            QP, KP, vslot = qkv_proj(xt, xtv, 0)

            def make_outproj_thunks(OT, b):
                fs = fpool.tile([128, 4, F], f32, name=f"fs{b}", tag="fs")
                thunks = []
                for j in range(4):

                    def th(j=j):
                        fp = pspool.tile(
                            [128, F], f32, name=f"fp{b}_{j}", tag="ps"
                        )
                        nc.tensor.matmul(
                            fp[:],
                            OT[0][:, 128 * j : 128 * (j + 1)],
                            wout_sb[:, 0, :],
                            start=True,
                            stop=False,
                        )
                        nc.tensor.matmul(
                            fp[:],
                            OT[1][:, 128 * j : 128 * (j + 1)],
                            wout_sb[:, 1, :],
                            start=False,
                            stop=True,
                        )
                        nc.vector.tensor_tensor(
                            fs[:, j, :], fp[:], bout_b[:], add
                        )
                        if j == 1 or j == 3:
                            nc.sync.dma_start(
                                out_d.rearrange(
                                    "(bb j p) f -> p (bb j) f", p=128, j=4
                                )[:, 4 * b + j - 1 : 4 * b + j + 1, :],
                                fs[:, j - 1 : j + 1, :],
                            )

                    thunks.append(th)
                return thunks

            def do_outproj(OT, b):
                for th in make_outproj_thunks(OT, b):
                    th()

            class Front:
                """scores + exp + strips for one head, emission split into
                weaveable pieces: mm(tt) emits one scores matmul; done(pp)
                emits the exp (and for pp=1 the strip multiplies)."""

                def __init__(self, h, QP, KP, gtag):
                    self.h, self.QP, self.KP = h, QP, KP
                    self.gtag = gtag
                    self.sp = {}
                    self.E1s = []
                    self.E2s = []

                def mm(self, tt):
                    h, et, hl = self.h, self.h // 2, self.h % 2
                    pp = tt // 2
                    if tt % 2 == 0:
                        self.sp[pp] = scpool.tile(
                            [128, 2, S], f32, name=f"sc{self.gtag}_{pp}",
                            tag="sc",
                        )
                    nc.tensor.matmul(
                        self.sp[pp][:, tt % 2, :],
                        self.KP[et][
                            64 * hl : 64 * hl + 64, 128 * tt : 128 * (tt + 1)
                        ],
                        self.QP[et][64 * hl : 64 * hl + 64, :],
                        start=True,
                        stop=True,
                        skip_group_check=True,
                    )

                def done(self, pp):
                    e1 = epool.tile(
                        [128, 2, S], bf16, name=f"e1{self.gtag}_{pp}", tag="e1"
                    )
                    nc.scalar.activation(
                        e1[:], self.sp[pp][:], Exp, scale=EXP_SCALE
                    )
                    self.E1s.append(e1[:, 0, :])
                    self.E1s.append(e1[:, 1, :])
                    if pp == 0:
                        return
                    # strips: in-place pe merge (E1 -> E1', DVE) and banded
                    # E2 = E1' * band (Pool). Strip tt covers s in
                    # [128tt-32, 128tt+160); local l in [lo, hi) clipped.
                    for tt in range(4):
                        lo = 32 if tt == 0 else 0
                        hi = 160 if tt == 3 else 192
                        reg = self.E1s[tt][
                            :, 128 * tt - 32 + lo : 128 * tt - 32 + hi
                        ]
                        nc.vector.tensor_tensor(
                            reg, reg, estrip_sb[:, lo:hi], mult
                        )
                        st = stpool.tile(
                            [128, 192], bf16, name=f"e2{self.gtag}_{tt}",
                            tag="e2",
                        )
                        nc.gpsimd.tensor_tensor(
                            st[:, lo:hi], reg, band_sb[:, lo:hi], mult
                        )
                        self.E2s.append(st)

                def run_all(self):
                    for tt in range(4):
                        self.mm(tt)
                        if tt % 2 == 1:
                            self.done(tt // 2)

            pending = []  # weaveable big-MM thunks (qkv / outproj chunks)

            def head_back(h, vslot, ET, E1s, E2s, nxt=None):
                """transposed AV + wide normalization + blend for head h."""
                et, hl = h // 2, h % 2
                # ---- transposed AV: per s-chunk st, out [128s, 65] =
                # E1'^T @ V~ (V~ = [V | ones] moving, N=65). Col 64 is the
                # per-s denominator -> wide per-partition reciprocal. ----
                PT1 = popool.tile([128, 4, 128], f32, tag="po")
                PT2 = popool.tile([128, 4, 128], f32, tag="po")
                vaug = vt_all[:, vslot]
                for st in range(4):
                    # weave big-stream matmuls (qkv/outproj chunks and the
                    # next head's scores) between the small AVT clusters to
                    # keep the PE array dense
                    if pending:
                        pending.pop(0)()
                    if nxt is not None:
                        nxt.mm(st)
                        if st % 2 == 1:
                            nxt.done(st // 2)
                    for tt in range(4):
                        nc.tensor.matmul(
                            PT1[:, st, 0:65],
                            E1s[tt][:, 128 * st : 128 * (st + 1)],
                            vaug[:, tt, h, :],
                            start=(tt == 0),
                            stop=(tt == 3),
                            skip_group_check=True,
                        )
                    # band: main strip tt=st covers the whole chunk; strip
                    # tt=st-1 covers s-subrange [0:32), tt=st+1 [96:128).
                    nc.tensor.matmul(
                        PT2[:, st, 0:65],
                        E2s[st][:, 32:160],
                        vaug[:, st, h, :],
                        start=True,
                        stop=False,
                        skip_group_check=True,
                    )
                    if st > 0:
                        nc.tensor.matmul(
                            PT2[0:32, st, 0:65],
                            E2s[st - 1][:, 160:192],
                            vaug[:, st - 1, h, :],
                            start=False,
                            stop=(st == 3),
                            skip_group_check=True,
                        )
                    if st < 3:
                        nc.tensor.matmul(
                            PT2[96:128, st, 0:65],
                            E2s[st + 1][:, 0:32],
                            vaug[:, st + 1, h, :],
                            start=False,
                            stop=True,
                            skip_group_check=True,
                            tile_position=(0, 96),
                        )

                # ---- normalization, all wide: [128,4] reciprocals,
                # free-dim-broadcast multiplies, blend into the ET tile ----
                rT1 = rpool.tile([128, 4], f32, tag="rt1")
                nc.vector.reciprocal(rT1[:], PT1[:, :, 64:65])
                rT2 = rpool.tile([128, 4], f32, tag="rt2")
                nc.vector.reciprocal(rT2[:], PT2[:, :, 64:65])
                tT1 = npool.tile([128, 4, 64], bf16, tag="t1")
                nc.vector.tensor_tensor(
                    tT1[:], PT1[:, :, 0:64],
                    rT1[:, :, None].broadcast_to((128, 4, 64)), mult,
                )
                tT2 = npool.tile([128, 4, 64], bf16, tag="t2")
                nc.vector.tensor_tensor(
                    tT2[:], PT2[:, :, 0:64],
                    rT2[:, :, None].broadcast_to((128, 4, 64)), mult,
                )
                nc.gpsimd.tensor_tensor(ET[et][:, :, hl, :], tT1[:], tT2[:], add)

            # ---- software-pipelined head stream: emit scores/exp/strips
            # for head g+1 BEFORE the AV/normalize of head g, so the PE
            # chews AV(g) while ACT/DVE/Pool produce head g+1's strips ----
            NG = BPC * H
            bctx = {0: (QP, KP, vslot)}  # per-batch (QP, KP, vslot)
            ET_all = {}
            OT_all = {}
            xt_next = None
            fronts = {}
            fronts[0] = Front(0, QP, KP, "g0")
            fronts[0].run_all()
            for g in range(NG):
                b, h = g // H, g % H
                if h == 0:
                    ET_all[b] = [
                        etpool.tile(
                            [128, 4, 2, 64], bf16, name=f"et{c}_{b}",
                            tag=f"et{c}",
                        )
                        for c in range(2)
                    ]
                    OT_all[b] = [None, None]
                    if b + 1 < BPC:
                        xt_next = load_xt(b + 1)
                nxt = None
                if g + 1 < NG:
                    QPf, KPf, _ = bctx[(g + 1) // H]
                    nxt = Front((g + 1) % H, QPf, KPf, f"g{g + 1}")
                    fronts[g + 1] = nxt
                fr = fronts.pop(g)
                _, _, vs = bctx[b]
                head_back(h, vs, ET_all[b], fr.E1s, fr.E2s, nxt=nxt)
                if h % 2 == 1:
                    # assemble OT[et] = ET[et].T via the DMA XBAR
                    et = h // 2
                    ot = otpool.tile(
                        [128, S], bf16, name=f"ot{et}_{b}", tag=f"ot{et}"
                    )
                    for st in range(4):
                        nc.sync.dma_start(
                            ot[:, 128 * st : 128 * (st + 1)],
                            ET_all[b][et][:, st, :, :],
                            transpose=True,
                        )
                    OT_all[b][et] = ot
                if h == 0 and b > 0:
                    # deferred out-proj of the previous batch
                    do_outproj(OT_all[b - 1], b - 1)
                if h == 1 and b + 1 < BPC:
                    # next batch's projections early
                    bctx[b + 1] = qkv_proj(*xt_next, b + 1)
            do_outproj(OT_all[BPC - 1], BPC - 1)

    nc.compile()
    return nc


_CACHE = {}
LAST_RESULTS = None


def prep_in_maps(inputs, Wq, bq, Wk, bk, Wv, bv, gamma, theta, Wout, bout):
    import ml_dtypes

    bfloat16 = ml_dtypes.bfloat16

    x = np.asarray(inputs, np.float32)
    Wq = np.asarray(Wq, np.float32)
    bq = np.asarray(bq, np.float32)
    Wk = np.asarray(Wk, np.float32)
    bk = np.asarray(bk, np.float32)
    Wv = np.asarray(Wv, np.float32)
    bv = np.asarray(bv, np.float32)
    Wout = np.asarray(Wout, np.float32)
    bout = np.asarray(bout, np.float32)
    gamma = float(np.asarray(gamma))
    theta = float(np.asarray(theta))

    # host-side prep. W{q,k} scaled by 32 for fp8 range; the projection
    # outputs are then 32x, scores 1024x -> compensated in EXP_SCALE
    # (with the softmax 1/sqrt(E)).
    WSC = 32.0
    fp8 = ml_dtypes.float8_e4m3
    wq_8 = (WSC * Wq).astype(fp8)
    wk_8 = (WSC * Wk).astype(fp8)
    wv_b = Wv.astype(bfloat16)
    qkb = (WSC * np.stack(
        [bq[:128], bq[128:], bk[:128], bk[128:]], axis=1
    )).astype(np.float32)  # [128, 4]
    bout_p = (bout + bv @ Wout).astype(np.float32).reshape(1, F)
    wout_h = (0.5 * Wout).astype(bfloat16)
    # strip coords: l = s - (128tt - 32); delta = t - s = p - l + 32.
    # estrip = exp(pe(delta)) (== 1 in bf16 beyond |delta|<=2);
    # bandmask = 1 where |delta| <= HALF_WIN else 0.
    p_i = np.arange(128)[:, None]
    l_i = np.arange(192)[None, :]
    delta = (p_i - l_i + 32).astype(np.float32)
    pe_val = np.exp(-np.abs(gamma * delta * delta - theta)).astype(np.float32)
    band = (np.abs(delta) <= HALF_WIN).astype(np.float32)
    estrip = np.exp(pe_val).astype(bfloat16)
    bandmask = band.astype(bfloat16)

    shared = {
        "wq": np.ascontiguousarray(wq_8),
        "wk": np.ascontiguousarray(wk_8),
        "wv": np.ascontiguousarray(wv_b),
        "wout": np.ascontiguousarray(wout_h),
        "qkbias": np.ascontiguousarray(qkb),
        "boutr": bout_p,
        "estrip": np.ascontiguousarray(estrip),
        "bandmask": np.ascontiguousarray(bandmask),
    }
    in_maps = []
    for c in range(NCORES):
        xc = x[c * BPC : (c + 1) * BPC].reshape(TOK, F)
        m = dict(shared)
        xct = xc.T
        m["xT"] = np.ascontiguousarray(xct.astype(fp8))
        m["xTv"] = np.ascontiguousarray(xct.astype(bfloat16))
        in_maps.append(m)
    return in_maps


def get_nc():
    if "nc" not in _CACHE:
        _CACHE["nc"] = _build()
    return _CACHE["nc"]


def kernel(inputs, Wq, bq, Wk, bk, Wv, bv, gamma, theta, Wout, bout):
    global LAST_RESULTS
    from concourse.bass_utils import run_bass_kernel_spmd

    in_maps = prep_in_maps(
        inputs, Wq, bq, Wk, bk, Wv, bv, gamma, theta, Wout, bout
    )
    nc = get_nc()
    res = run_bass_kernel_spmd(nc, in_maps, core_ids=list(range(NCORES)))
    LAST_RESULTS = res
    out = np.concatenate(
        [res.results[c]["out"].reshape(BPC, S, F) for c in range(NCORES)], axis=0
    )
    return out


# revision 39
# speedup vs baseline: 1.3961x; 1.0488x over previous
"""Trainium2 Bass kernel for nn_ContextAttention (sparse_attention).

Math (per batch b):
  q = (x @ Wq + bq) / 16 ; k = x @ Wk + bk ; v0 = x @ Wv   (bv folded into bout)
  scoresT[t,s] = sum_d kT[d,t] qT[d,s]
  E1 = exp(scoresT); E1 *= exp(pe) on the 192-wide diagonal strip, in place
      (exp(pe) == 1 in bf16 beyond |t-s|<=2, so the strip covers pe exactly)
  E2 = E1' * band(|t-s|<=32)   (banded strips only)
  o1T[d,s] = sum_t V~[t,d] E1'[t,s] with V~=[V|1] -> row 64 = denominator d1
  o2T      = banded AV of the E2 strips (ones col gives band denominator)
  OT = o1T/d1 + o2T/d2   (x0.5 folded into Wout)
  out = OT.T @ (0.5*Wout) + (bv @ Wout + bout)

Sharding: data-parallel over batch across 8 cores (8 batches each). No
collectives.

v3 vs v2 (the 1.0 ms baseline):
  - pe correction merged INTO E1 in place -> o1 is a plain dense AV
    (removes the 10 correction matmuls per head).
  - normalization: one reciprocal_approx_fast per head on the merged
    [1,2,512] denominator rows (was 2x 3.3us iterative reciprocals =
    212us of the 1ms), one merged partition_broadcast, one merged norm
    multiply, one blend add.
  - o1/o2 live in one [128,2,512] psum tile per head.
  - V ones/zero columns in a persistent manually double-buffered const
    tile (no per-batch memsets).
"""

import sys

sys.path.insert(0, "/opt/trn_rl_repo")

import numpy as np

B, S, F, E, H, DH = 64, 512, 512, 256, 4, 64
HALF_WIN = 32
SCALE = 16.0  # EMBED ** 0.5
NCORES = 8
BPC = B // NCORES  # batches per core
TOK = BPC * S  # tokens per core


def _build():
    import concourse.bacc as bacc
    import concourse.tile as tile
    from concourse import mybir

    f32 = mybir.dt.float32
    f32r = mybir.dt.float32r
    bf16 = mybir.dt.bfloat16
    fp8 = mybir.dt.float8e4
    DR = mybir.MatmulPerfMode.DoubleRow
    # x is fp8; W{q,k} are fp8 pre-scaled by 32 (avoids fp8 subnormals).
    # scores psum = (32k)(32q) = 1024 * k.q ; softmax scale 1/16 folds in too.
    EXP_SCALE = 1.0 / (1024.0 * 16.0)
    Copy = mybir.ActivationFunctionType.Copy
    Exp = mybir.ActivationFunctionType.Exp
    mult = mybir.AluOpType.mult
    add = mybir.AluOpType.add

    nc = bacc.Bacc("TRN2", target_bir_lowering=False, debug=False)

    xT = nc.dram_tensor("xT", [F, TOK], fp8, kind="ExternalInput")
    wq_d = nc.dram_tensor("wq", [F, E], fp8, kind="ExternalInput")
    wk_d = nc.dram_tensor("wk", [F, E], fp8, kind="ExternalInput")
    wv_d = nc.dram_tensor("wv", [F, E], bf16, kind="ExternalInput")
    xTv = nc.dram_tensor("xTv", [F, TOK], bf16, kind="ExternalInput")
    wout_d = nc.dram_tensor("wout", [E, F], bf16, kind="ExternalInput")
    qkb_d = nc.dram_tensor("qkbias", [128, 4], f32, kind="ExternalInput")
    bout_d = nc.dram_tensor("boutr", [1, F], f32, kind="ExternalInput")
    estrip_d = nc.dram_tensor("estrip", [128, 192], bf16, kind="ExternalInput")
    band_d = nc.dram_tensor("bandmask", [128, 192], bf16, kind="ExternalInput")
    out_d = nc.dram_tensor("out", [TOK, F], f32, kind="ExternalOutput")

    with tile.TileContext(nc) as tc:
        with (
            tc.tile_pool(name="const", bufs=1) as const,
            tc.tile_pool(name="xt", bufs=2) as xpool,
            tc.tile_pool(name="qk", bufs=2) as qkpool,
            tc.tile_pool(name="ee", bufs=16) as epool,
            tc.tile_pool(name="st", bufs=16) as stpool,
            tc.tile_pool(name="rr", bufs=3) as rpool,
            tc.tile_pool(name="nn", bufs=3) as npool,
            tc.tile_pool(name="et", bufs=2) as etpool,
            tc.tile_pool(name="ot", bufs=2) as otpool,
            tc.tile_pool(name="ff", bufs=2) as fpool,
            tc.tile_pool(name="ps", bufs=2, space="PSUM") as pspool,
            tc.tile_pool(name="sc", bufs=3, space="PSUM") as scpool,
            tc.tile_pool(name="po", bufs=3, space="PSUM") as popool,
        ):
            # ---- persistent constants (spread across engine queues so the
            # critical path to the first matmul is short) ----
            wq_sb = const.tile([128, 4, E], fp8, tag="wq")
            nc.sync.dma_start(
                wq_sb[:], wq_d.rearrange("(c p) e -> p c e", p=128)
            )
            wk_sb = const.tile([128, 4, E], fp8, tag="wk")
            nc.scalar.dma_start(
                wk_sb[:], wk_d.rearrange("(c p) e -> p c e", p=128)
            )
            wv_sb = const.tile([128, 4, E], bf16, tag="wv")
            nc.gpsimd.dma_start(
                wv_sb[:], wv_d.rearrange("(c p) e -> p c e", p=128)
            )
            wout_sb = const.tile([128, 2, F], bf16, tag="wout")
            nc.gpsimd.dma_start(
                wout_sb[:], wout_d.rearrange("(c p) e -> p c e", p=128)
            )
            estrip_sb = const.tile([128, 192], bf16, tag="estrip")
            nc.gpsimd.dma_start(estrip_sb[:], estrip_d[:, :])
            qkb_sb = const.tile([128, 4], f32, tag="qkb")
            nc.scalar.dma_start(qkb_sb[:], qkb_d[:, :])
            band_sb = const.tile([128, 192], bf16, tag="band")
            nc.scalar.dma_start(band_sb[:], band_d[:, :])
            bout_row = const.tile([1, F], f32, tag="boutrow")
            nc.gpsimd.dma_start(bout_row[:], bout_d[0:1, :])
            bout_b = const.tile([128, F], f32, tag="boutb")
            nc.gpsimd.partition_broadcast(bout_b[:], bout_row[:])
            # V tiles: [128t, slot, ttile, head, 128] with col 64 = ones
            # (denominator) and cols 65:128 = 0 (keeps M=128 so FWL stays
            # on). Ones/zeros written ONCE; per-batch V-copies only touch
            # cols 0:64.
            vt_all = const.tile([128, 2, 4, 4, 65], bf16, tag="vt")
            nc.gpsimd.memset(vt_all[:, :, :, :, 64:65], 1.0)

            def load_xt(b):
                xt = xpool.tile([128, 4, S], fp8, tag="xt")
                nc.sync.dma_start(
                    xt[:],
                    xT.rearrange("(c p) t -> p c t", p=128)[
                        :, :, 512 * b : 512 * (b + 1)
                    ],
                )
                xtv = xpool.tile([128, 4, S], bf16, tag="xtv")
                nc.sync.dma_start(
                    xtv[:],
                    xTv.rearrange("(c p) t -> p c t", p=128)[
                        :, :, 512 * b : 512 * (b + 1)
                    ],
                )
                return xt, xtv

            def make_qkv_thunks(xt, xtv, b):
                """Q/K/V projections for batch b as 8 weaveable thunks.
                Output tiles are allocated eagerly so callers can reference
                them before the thunks have emitted."""
                QP, KP = [], []
                slot = b % 2
                thunks = []
                for et in range(2):
                    for lst, w_sb, bcol in ((QP, wq_sb, 0), (KP, wk_sb, 2)):
                        t = qkpool.tile(
                            [128, S], bf16, name=f"qk{b}_{bcol}_{et}",
                            tag=f"{'q' if bcol == 0 else 'k'}p{et}",
                        )
                        lst.append(t)

                        def th(et=et, w_sb=w_sb, bcol=bcol, t=t):
                            ps = pspool.tile(
                                [128, S], f32, name=f"ps{b}_{bcol}_{et}",
                                tag="ps",
                            )
                            for kc in range(2):
                                nc.tensor.matmul(
                                    ps[:],
                                    w_sb[
                                        :, 2 * kc : 2 * kc + 2,
                                        128 * et : 128 * (et + 1),
                                    ],
                                    xt[:, 2 * kc : 2 * kc + 2, :],
                                    start=(kc == 0),
                                    stop=(kc == 1),
                                    perf_mode=DR,
                                )
                            nc.scalar.add(
                                t[:], ps[:],
                                qkb_sb[:, bcol + et : bcol + et + 1],
                            )

                        thunks.append(th)
                for j in range(4):

                    def th(j=j):
                        ps = pspool.tile(
                            [128, E], f32, name=f"psv{b}_{j}", tag="ps"
                        )
                        for kc in range(4):
                            nc.tensor.matmul(
                                ps[:],
                                xtv[:, kc, 128 * j : 128 * (j + 1)],
                                wv_sb[:, kc, :],
                                start=(kc == 0),
                                stop=(kc == 3),
                            )
                        nc.vector.tensor_copy(
                            vt_all[:, slot, j, :, 0:64],
                            ps.rearrange("p (h x) -> p h x", x=64),
                        )

                    thunks.append(th)
                return thunks, (QP, KP, slot)

            def qkv_proj(xt, xtv, b):
                thunks, ctx = make_qkv_thunks(xt, xtv, b)
                for th in thunks:
                    th()
                return ctx

            xt, xtv = load_xt(0)
            QP, KP, vslot = qkv_proj(xt, xtv, 0)

            def make_outproj_thunks(OT, b):
                fs = fpool.tile([128, 4, F], f32, name=f"fs{b}", tag="fs")
                thunks = []
                for j in range(4):

                    def th(j=j):
                        fp = pspool.tile(
                            [128, F], f32, name=f"fp{b}_{j}", tag="ps"
                        )
                        nc.tensor.matmul(
                            fp[:],
                            OT[0][:, 128 * j : 128 * (j + 1)],
                            wout_sb[:, 0, :],
                            start=True,
                            stop=False,
                        )
                        nc.tensor.matmul(
                            fp[:],
                            OT[1][:, 128 * j : 128 * (j + 1)],
                            wout_sb[:, 1, :],
                            start=False,
                            stop=True,
                        )
                        nc.vector.tensor_tensor(
                            fs[:, j, :], fp[:], bout_b[:], add
                        )
                        if j == 1 or j == 3:
                            nc.sync.dma_start(
                                out_d.rearrange(
                                    "(bb j p) f -> p (bb j) f", p=128, j=4
                                )[:, 4 * b + j - 1 : 4 * b + j + 1, :],
                                fs[:, j - 1 : j + 1, :],
                            )

                    thunks.append(th)
                return thunks

            def do_outproj(OT, b):
                for th in make_outproj_thunks(OT, b):
                    th()

            class Front:
                """scores + exp + strips for one head, emission split into
                weaveable per-tt pieces. Two heads of an et-pair issue their
                mm(tt) back to back: the row halves (hl=0 -> rows 0:64,
                hl=1 -> 64:128) land in disjoint PE row groups and stream
                concurrently."""

                def __init__(self, h, QP, KP, gtag):
                    self.h, self.QP, self.KP = h, QP, KP
                    self.gtag = gtag
                    self.sc = [None] * 4
                    self.E1s = [None] * 4
                    self.E2s = [None] * 4

                def mm(self, tt):
                    h, et, hl = self.h, self.h // 2, self.h % 2
                    sc = scpool.tile(
                        [128, S], f32, name=f"sc{self.gtag}_{tt}", tag="sc"
                    )
                    self.sc[tt] = sc
                    nc.tensor.matmul(
                        sc[:],
                        self.KP[et][
                            64 * hl : 64 * hl + 64, 128 * tt : 128 * (tt + 1)
                        ],
                        self.QP[et][64 * hl : 64 * hl + 64, :],
                        start=True,
                        stop=True,
                        skip_group_check=True,
                    )

                def expstrip(self, tt):
                    # exp, then in-place pe merge (E1 -> E1', DVE) and the
                    # banded E2 = E1' * band (Pool). Strip tt covers s in
                    # [128tt-32, 128tt+160); local l in [lo, hi) clipped.
                    e1 = epool.tile(
                        [128, S], bf16, name=f"e1{self.gtag}_{tt}", tag="e1"
                    )
                    nc.scalar.activation(
                        e1[:], self.sc[tt][:], Exp, scale=EXP_SCALE
                    )
                    self.E1s[tt] = e1
                    lo = 32 if tt == 0 else 0
                    hi = 160 if tt == 3 else 192
                    reg = e1[:, 128 * tt - 32 + lo : 128 * tt - 32 + hi]
                    nc.vector.tensor_tensor(reg, reg, estrip_sb[:, lo:hi], mult)
                    st = stpool.tile(
                        [128, 192], bf16, name=f"e2{self.gtag}_{tt}", tag="e2"
                    )
                    nc.gpsimd.tensor_tensor(
                        st[:, lo:hi], reg, band_sb[:, lo:hi], mult
                    )
                    self.E2s[tt] = st

                def run_all(self):
                    for tt in range(4):
                        self.mm(tt)
                        self.expstrip(tt)

            pending = []  # weaveable big-MM thunks (qkv / outproj chunks)

            def head_back(h, vslot, ET, E1s, E2s, nxt=None):
                """transposed AV + wide normalization + blend for head h."""
                et, hl = h // 2, h % 2
                # ---- transposed AV: per s-chunk st, out [128s, 65] =
                # E1'^T @ V~ (V~ = [V | ones] moving, N=65). Col 64 is the
                # per-s denominator -> wide per-partition reciprocal. ----
                PT1 = popool.tile([128, 4, 128], f32, tag="po")
                PT2 = popool.tile([128, 4, 128], f32, tag="po")
                vaug = vt_all[:, vslot]
                for st in range(4):
                    # weave the next head-pair's scores (row-group-paired
                    # matmuls stream concurrently) between the small AVT
                    # clusters to keep the PE array dense
                    if nxt is not None:
                        fa, fb = nxt
                        fa.mm(st)
                        fb.mm(st)
                        fa.expstrip(st)
                        fb.expstrip(st)
                    for tt in range(4):
                        nc.tensor.matmul(
                            PT1[:, st, 0:65],
                            E1s[tt][:, 128 * st : 128 * (st + 1)],
                            vaug[:, tt, h, :],
                            start=(tt == 0),
                            stop=(tt == 3),
                            skip_group_check=True,
                        )
                    # band: main strip tt=st covers the whole chunk; strip
                    # tt=st-1 covers s-subrange [0:32), tt=st+1 [96:128).
                    nc.tensor.matmul(
                        PT2[:, st, 0:65],
                        E2s[st][:, 32:160],
                        vaug[:, st, h, :],
                        start=True,
                        stop=False,
                        skip_group_check=True,
                    )
                    if st > 0:
                        nc.tensor.matmul(
                            PT2[0:32, st, 0:65],
                            E2s[st - 1][:, 160:192],
                            vaug[:, st - 1, h, :],
                            start=False,
                            stop=(st == 3),
                            skip_group_check=True,
                        )
                    if st < 3:
                        nc.tensor.matmul(
                            PT2[96:128, st, 0:65],
                            E2s[st + 1][:, 0:32],
                            vaug[:, st + 1, h, :],
                            start=False,
                            stop=True,
                            skip_group_check=True,
                            tile_position=(0, 96),
                        )

                # ---- normalization, all wide: [128,4] reciprocals,
                # free-dim-broadcast multiplies, blend into the ET tile ----
                rT1 = rpool.tile([128, 4], f32, tag="rt1")
                nc.vector.reciprocal(rT1[:], PT1[:, :, 64:65])
                rT2 = rpool.tile([128, 4], f32, tag="rt2")
                nc.vector.reciprocal(rT2[:], PT2[:, :, 64:65])
                tT1 = npool.tile([128, 4, 64], bf16, tag="t1")
                nc.vector.tensor_tensor(
                    tT1[:], PT1[:, :, 0:64],
                    rT1[:, :, None].broadcast_to((128, 4, 64)), mult,
                )
                tT2 = npool.tile([128, 4, 64], bf16, tag="t2")
                nc.vector.tensor_tensor(
                    tT2[:], PT2[:, :, 0:64],
                    rT2[:, :, None].broadcast_to((128, 4, 64)), mult,
                )
                nc.gpsimd.tensor_tensor(ET[et][:, :, hl, :], tT1[:], tT2[:], add)

            # ---- software-pipelined head stream: emit scores/exp/strips
            # for head g+1 BEFORE the AV/normalize of head g, so the PE
            # chews AV(g) while ACT/DVE/Pool produce head g+1's strips ----
            NG = BPC * H
            bctx = {0: (QP, KP, vslot)}  # per-batch (QP, KP, vslot)
            ET_all = {}
            OT_all = {}
            xt_next = None
            fronts = {}
            fronts[0] = Front(0, QP, KP, "g0")
            fronts[1] = Front(1, QP, KP, "g1")
            for tt in range(4):
                for f in (fronts[0], fronts[1]):
                    f.mm(tt)
                for f in (fronts[0], fronts[1]):
                    f.expstrip(tt)
            for g in range(NG):
                b, h = g // H, g % H
                if h == 0:
                    ET_all[b] = [
                        etpool.tile(
                            [128, 4, 2, 64], bf16, name=f"et{c}_{b}",
                            tag=f"et{c}",
                        )
                        for c in range(2)
                    ]
                    OT_all[b] = [None, None]
                    if b + 1 < BPC:
                        xt_next = load_xt(b + 1)
                nxt = None
                if h % 2 == 0 and g + 2 < NG:
                    # prepare the NEXT et-pair's fronts, woven into this
                    # back as row-group-paired score matmuls
                    QPf, KPf, _ = bctx[(g + 2) // H]
                    fa = Front((g + 2) % H, QPf, KPf, f"g{g + 2}")
                    fb = Front((g + 3) % H, QPf, KPf, f"g{g + 3}")
                    fronts[g + 2], fronts[g + 3] = fa, fb
                    nxt = (fa, fb)
                fr = fronts.pop(g)
                _, _, vs = bctx[b]
                head_back(h, vs, ET_all[b], fr.E1s, fr.E2s, nxt=nxt)
                if h % 2 == 1:
                    # assemble OT[et] = ET[et].T via the DMA XBAR
                    et = h // 2
                    ot = otpool.tile(
                        [128, S], bf16, name=f"ot{et}_{b}", tag=f"ot{et}"
                    )
                    for st in range(4):
                        nc.sync.dma_start(
                            ot[:, 128 * st : 128 * (st + 1)],
                            ET_all[b][et][:, st, :, :],
                            transpose=True,
                        )
                    OT_all[b][et] = ot
                if h == 0 and b > 0:
                    # deferred out-proj of the previous batch
                    do_outproj(OT_all[b - 1], b - 1)
                if h == 1 and b + 1 < BPC:
                    # next batch's projections early
                    bctx[b + 1] = qkv_proj(*xt_next, b + 1)
            do_outproj(OT_all[BPC - 1], BPC - 1)

    nc.compile()
    return nc


_CACHE = {}
LAST_RESULTS = None


def prep_in_maps(inputs, Wq, bq, Wk, bk, Wv, bv, gamma, theta, Wout, bout):
    import ml_dtypes

    bfloat16 = ml_dtypes.bfloat16

    x = np.asarray(inputs, np.float32)
    Wq = np.asarray(Wq, np.float32)
    bq = np.asarray(bq, np.float32)
    Wk = np.asarray(Wk, np.float32)
    bk = np.asarray(bk, np.float32)
    Wv = np.asarray(Wv, np.float32)
    bv = np.asarray(bv, np.float32)
    Wout = np.asarray(Wout, np.float32)
    bout = np.asarray(bout, np.float32)
    gamma = float(np.asarray(gamma))
    theta = float(np.asarray(theta))

    # host-side prep. W{q,k} scaled by 32 for fp8 range; the projection
    # outputs are then 32x, scores 1024x -> compensated in EXP_SCALE
    # (with the softmax 1/sqrt(E)).
    WSC = 32.0
    fp8 = ml_dtypes.float8_e4m3
    wq_8 = (WSC * Wq).astype(fp8)
    wk_8 = (WSC * Wk).astype(fp8)
    wv_b = Wv.astype(bfloat16)
    qkb = (WSC * np.stack(
        [bq[:128], bq[128:], bk[:128], bk[128:]], axis=1
    )).astype(np.float32)  # [128, 4]
    bout_p = (bout + bv @ Wout).astype(np.float32).reshape(1, F)
    wout_h = (0.5 * Wout).astype(bfloat16)
    # strip coords: l = s - (128tt - 32); delta = t - s = p - l + 32.
    # estrip = exp(pe(delta)) (== 1 in bf16 beyond |delta|<=2);
    # bandmask = 1 where |delta| <= HALF_WIN else 0.
    p_i = np.arange(128)[:, None]
    l_i = np.arange(192)[None, :]
    delta = (p_i - l_i + 32).astype(np.float32)
    pe_val = np.exp(-np.abs(gamma * delta * delta - theta)).astype(np.float32)
    band = (np.abs(delta) <= HALF_WIN).astype(np.float32)
    estrip = np.exp(pe_val).astype(bfloat16)
    bandmask = band.astype(bfloat16)

    shared = {
        "wq": np.ascontiguousarray(wq_8),
        "wk": np.ascontiguousarray(wk_8),
        "wv": np.ascontiguousarray(wv_b),
        "wout": np.ascontiguousarray(wout_h),
        "qkbias": np.ascontiguousarray(qkb),
        "boutr": bout_p,
        "estrip": np.ascontiguousarray(estrip),
        "bandmask": np.ascontiguousarray(bandmask),
    }
    in_maps = []
    for c in range(NCORES):
        xc = x[c * BPC : (c + 1) * BPC].reshape(TOK, F)
        m = dict(shared)
        xct = xc.T
        m["xT"] = np.ascontiguousarray(xct.astype(fp8))
        m["xTv"] = np.ascontiguousarray(xct.astype(bfloat16))
        in_maps.append(m)
    return in_maps


def get_nc():
    if "nc" not in _CACHE:
        _CACHE["nc"] = _build()
    return _CACHE["nc"]


def kernel(inputs, Wq, bq, Wk, bk, Wv, bv, gamma, theta, Wout, bout):
    global LAST_RESULTS
    from concourse.bass_utils import run_bass_kernel_spmd

    in_maps = prep_in_maps(
        inputs, Wq, bq, Wk, bk, Wv, bv, gamma, theta, Wout, bout
    )
    nc = get_nc()
    res = run_bass_kernel_spmd(nc, in_maps, core_ids=list(range(NCORES)))
    LAST_RESULTS = res
    out = np.concatenate(
        [res.results[c]["out"].reshape(BPC, S, F) for c in range(NCORES)], axis=0
    )
    return out


# revision 41
# speedup vs baseline: 1.4408x; 1.0320x over previous
"""Trainium2 Bass kernel for nn_ContextAttention (sparse_attention).

Math (per batch b):
  q = (x @ Wq + bq) / 16 ; k = x @ Wk + bk ; v0 = x @ Wv   (bv folded into bout)
  scoresT[t,s] = sum_d kT[d,t] qT[d,s]
  E1 = exp(scoresT); E1 *= exp(pe) on the 192-wide diagonal strip, in place
      (exp(pe) == 1 in bf16 beyond |t-s|<=2, so the strip covers pe exactly)
  E2 = E1' * band(|t-s|<=32)   (banded strips only)
  o1T[d,s] = sum_t V~[t,d] E1'[t,s] with V~=[V|1] -> row 64 = denominator d1
  o2T      = banded AV of the E2 strips (ones col gives band denominator)
  OT = o1T/d1 + o2T/d2   (x0.5 folded into Wout)
  out = OT.T @ (0.5*Wout) + (bv @ Wout + bout)

Sharding: data-parallel over batch across 8 cores (8 batches each). No
collectives.

Design (v9, ~248 us/core vs the 1.0 ms v2 baseline):
  - pe correction merged INTO E1 in place -> o1 is a plain dense AV
    (no correction matmuls).
  - TRANSPOSED AV: per s-chunk, out [128s, 65] = E1'^T @ [V|1], so the
    softmax denominators land as per-partition columns. Normalization is
    then a [128,4] iterative reciprocal (~190ns) + free-dim-broadcast
    multiplies + blend -- this replaced 212us of single-lane [1,512]
    reciprocals plus 66us of partition_broadcasts in the v2 baseline.
    (reciprocal_approx_fast and all custom-DVE ops return garbage on
    this runtime -- ucode tables are not loaded -- so only the regular
    iterative reciprocal is safe.)
  - OT ([e,s] for the out-proj) is assembled from the [s,e] blend tiles
    by DMA-XBAR transposes (engine-free).
  - scores in bf16 (f32r lowers to fp32-HIGH multi-pass on HW: 1.5x
    slower), with the two heads of an et-pair issued as adjacent
    row-group-tiled matmuls (rows 0:64 / 64:128) that stream
    concurrently; 1-bank score psums with per-tt exps.
  - software-pipelined emission: head-pair p+1's scores/exp/strips are
    woven between head-pair p's AVT matmul clusters, qkv projections of
    batch b+1 and out-proj of batch b-1 fill the slot boundaries.
  - V ones columns in a persistent manually double-buffered const tile.
"""

import sys

sys.path.insert(0, "/opt/trn_rl_repo")

import numpy as np

B, S, F, E, H, DH = 64, 512, 512, 256, 4, 64
HALF_WIN = 32
SCALE = 16.0  # EMBED ** 0.5
NCORES = 8
BPC = B // NCORES  # batches per core
TOK = BPC * S  # tokens per core


def _build():
    import concourse.bacc as bacc
    import concourse.tile as tile
    from concourse import mybir

    f32 = mybir.dt.float32
    f32r = mybir.dt.float32r
    bf16 = mybir.dt.bfloat16
    fp8 = mybir.dt.float8e4
    DR = mybir.MatmulPerfMode.DoubleRow
    # x is fp8; W{q,k} are fp8 pre-scaled by 32 (avoids fp8 subnormals).
    # scores psum = (32k)(32q) = 1024 * k.q ; softmax scale 1/16 folds in too.
    EXP_SCALE = 1.0 / (1024.0 * 16.0)
    Copy = mybir.ActivationFunctionType.Copy
    Exp = mybir.ActivationFunctionType.Exp
    mult = mybir.AluOpType.mult
    add = mybir.AluOpType.add

    nc = bacc.Bacc("TRN2", target_bir_lowering=False, debug=False)

    xT = nc.dram_tensor("xT", [F, TOK], fp8, kind="ExternalInput")
    wq_d = nc.dram_tensor("wq", [F, E], fp8, kind="ExternalInput")
    wk_d = nc.dram_tensor("wk", [F, E], fp8, kind="ExternalInput")
    wv_d = nc.dram_tensor("wv", [F, E], bf16, kind="ExternalInput")
    xTv = nc.dram_tensor("xTv", [F, TOK], bf16, kind="ExternalInput")
    wout_d = nc.dram_tensor("wout", [E, F], bf16, kind="ExternalInput")
    qkb_d = nc.dram_tensor("qkbias", [128, 4], f32, kind="ExternalInput")
    bout_d = nc.dram_tensor("boutr", [1, F], f32, kind="ExternalInput")
    estrip_d = nc.dram_tensor("estrip", [128, 192], bf16, kind="ExternalInput")
    band_d = nc.dram_tensor("bandmask", [128, 192], bf16, kind="ExternalInput")
    out_d = nc.dram_tensor("out", [TOK, F], f32, kind="ExternalOutput")

    with tile.TileContext(nc) as tc:
        with (
            tc.tile_pool(name="const", bufs=1) as const,
            tc.tile_pool(name="xt", bufs=3) as xpool,
            tc.tile_pool(name="qk", bufs=2) as qkpool,
            tc.tile_pool(name="ee", bufs=16) as epool,
            tc.tile_pool(name="st", bufs=16) as stpool,
            tc.tile_pool(name="rr", bufs=3) as rpool,
            tc.tile_pool(name="nn", bufs=3) as npool,
            tc.tile_pool(name="et", bufs=2) as etpool,
            tc.tile_pool(name="ot", bufs=2) as otpool,
            tc.tile_pool(name="ff", bufs=2) as fpool,
            tc.tile_pool(name="ps", bufs=2, space="PSUM") as pspool,
            tc.tile_pool(name="sc", bufs=3, space="PSUM") as scpool,
            tc.tile_pool(name="po", bufs=3, space="PSUM") as popool,
        ):
            # ---- persistent constants (spread across engine queues so the
            # critical path to the first matmul is short) ----
            wq_sb = const.tile([128, 4, E], fp8, tag="wq")
            nc.sync.dma_start(
                wq_sb[:], wq_d.rearrange("(c p) e -> p c e", p=128)
            )
            wk_sb = const.tile([128, 4, E], fp8, tag="wk")
            nc.scalar.dma_start(
                wk_sb[:], wk_d.rearrange("(c p) e -> p c e", p=128)
            )
            wv_sb = const.tile([128, 4, E], bf16, tag="wv")
            nc.gpsimd.dma_start(
                wv_sb[:], wv_d.rearrange("(c p) e -> p c e", p=128)
            )
            wout_sb = const.tile([128, 2, F], bf16, tag="wout")
            nc.gpsimd.dma_start(
                wout_sb[:], wout_d.rearrange("(c p) e -> p c e", p=128)
            )
            estrip_sb = const.tile([128, 192], bf16, tag="estrip")
            nc.gpsimd.dma_start(estrip_sb[:], estrip_d[:, :])
            qkb_sb = const.tile([128, 4], f32, tag="qkb")
            nc.scalar.dma_start(qkb_sb[:], qkb_d[:, :])
            band_sb = const.tile([128, 192], bf16, tag="band")
            nc.scalar.dma_start(band_sb[:], band_d[:, :])
            bout_row = const.tile([1, F], f32, tag="boutrow")
            nc.gpsimd.dma_start(bout_row[:], bout_d[0:1, :])
            bout_b = const.tile([128, F], f32, tag="boutb")
            nc.gpsimd.partition_broadcast(bout_b[:], bout_row[:])
            # V tiles: [128t, slot, ttile, head, 128] with col 64 = ones
            # (denominator) and cols 65:128 = 0 (keeps M=128 so FWL stays
            # on). Ones/zeros written ONCE; per-batch V-copies only touch
            # cols 0:64.
            vt_all = const.tile([128, 2, 4, 4, 65], bf16, tag="vt")
            nc.gpsimd.memset(vt_all[:, :, :, :, 64:65], 1.0)

            def load_xt(b):
                xt = xpool.tile([128, 4, S], fp8, tag="xt")
                nc.sync.dma_start(
                    xt[:],
                    xT.rearrange("(c p) t -> p c t", p=128)[
                        :, :, 512 * b : 512 * (b + 1)
                    ],
                )
                xtv = xpool.tile([128, 4, S], bf16, tag="xtv")
                nc.sync.dma_start(
                    xtv[:],
                    xTv.rearrange("(c p) t -> p c t", p=128)[
                        :, :, 512 * b : 512 * (b + 1)
                    ],
                )
                return xt, xtv

            def make_qkv_thunks(xt, xtv, b):
                """Q/K/V projections for batch b as 8 weaveable thunks.
                Output tiles are allocated eagerly so callers can reference
                them before the thunks have emitted."""
                QP, KP = [], []
                slot = b % 2
                thunks = []
                for et in range(2):
                    for lst, w_sb, bcol in ((QP, wq_sb, 0), (KP, wk_sb, 2)):
                        t = qkpool.tile(
                            [128, S], bf16, name=f"qk{b}_{bcol}_{et}",
                            tag=f"{'q' if bcol == 0 else 'k'}p{et}",
                        )
                        lst.append(t)

                        def th(et=et, w_sb=w_sb, bcol=bcol, t=t):
                            ps = pspool.tile(
                                [128, S], f32, name=f"ps{b}_{bcol}_{et}",
                                tag="ps",
                            )
                            for kc in range(2):
                                nc.tensor.matmul(
                                    ps[:],
                                    w_sb[
                                        :, 2 * kc : 2 * kc + 2,
                                        128 * et : 128 * (et + 1),
                                    ],
                                    xt[:, 2 * kc : 2 * kc + 2, :],
                                    start=(kc == 0),
                                    stop=(kc == 1),
                                    perf_mode=DR,
                                )
                            nc.scalar.add(
                                t[:], ps[:],
                                qkb_sb[:, bcol + et : bcol + et + 1],
                            )

                        thunks.append(th)
                for j in range(4):

                    def th(j=j):
                        ps = pspool.tile(
                            [128, E], f32, name=f"psv{b}_{j}", tag="ps"
                        )
                        for kc in range(4):
                            nc.tensor.matmul(
                                ps[:],
                                xtv[:, kc, 128 * j : 128 * (j + 1)],
                                wv_sb[:, kc, :],
                                start=(kc == 0),
                                stop=(kc == 3),
                            )
                        nc.vector.tensor_copy(
                            vt_all[:, slot, j, :, 0:64],
                            ps.rearrange("p (h x) -> p h x", x=64),
                        )

                    thunks.append(th)
                return thunks, (QP, KP, slot)

            def qkv_proj(xt, xtv, b):
                thunks, ctx = make_qkv_thunks(xt, xtv, b)
                for th in thunks:
                    th()
                return ctx

            xts = {0: load_xt(0)}
            QP, KP, vslot = qkv_proj(*xts[0], 0)
            xts[1] = load_xt(1)

            def make_outproj_thunks(OT, b):
                fs = fpool.tile([128, 4, F], f32, name=f"fs{b}", tag="fs")
                thunks = []
                for j in range(4):

                    def th(j=j):
                        fp = pspool.tile(
                            [128, F], f32, name=f"fp{b}_{j}", tag="ps"
                        )
                        nc.tensor.matmul(
                            fp[:],
                            OT[0][:, 128 * j : 128 * (j + 1)],
                            wout_sb[:, 0, :],
                            start=True,
                            stop=False,
                        )
                        nc.tensor.matmul(
                            fp[:],
                            OT[1][:, 128 * j : 128 * (j + 1)],
                            wout_sb[:, 1, :],
                            start=False,
                            stop=True,
                        )
                        nc.vector.tensor_tensor(
                            fs[:, j, :], fp[:], bout_b[:], add
                        )
                        if j == 1 or j == 3:
                            nc.sync.dma_start(
                                out_d.rearrange(
                                    "(bb j p) f -> p (bb j) f", p=128, j=4
                                )[:, 4 * b + j - 1 : 4 * b + j + 1, :],
                                fs[:, j - 1 : j + 1, :],
                            )

                    thunks.append(th)
                return thunks

            def do_outproj(OT, b):
                for th in make_outproj_thunks(OT, b):
                    th()

            class Front:
                """scores + exp + strips for one head, emission split into
                weaveable per-tt pieces. Two heads of an et-pair issue their
                mm(tt) back to back: the row halves (hl=0 -> rows 0:64,
                hl=1 -> 64:128) land in disjoint PE row groups and stream
                concurrently."""

                def __init__(self, h, QP, KP, gtag):
                    self.h, self.QP, self.KP = h, QP, KP
                    self.gtag = gtag
                    self.sc = [None] * 4
                    self.E1s = [None] * 4
                    self.E2s = [None] * 4

                def mm(self, tt):
                    h, et, hl = self.h, self.h // 2, self.h % 2
                    sc = scpool.tile(
                        [128, S], f32, name=f"sc{self.gtag}_{tt}", tag="sc"
                    )
                    self.sc[tt] = sc
                    nc.tensor.matmul(
                        sc[:],
                        self.KP[et][
                            64 * hl : 64 * hl + 64, 128 * tt : 128 * (tt + 1)
                        ],
                        self.QP[et][64 * hl : 64 * hl + 64, :],
                        start=True,
                        stop=True,
                        skip_group_check=True,
                    )

                def expstrip(self, tt):
                    # exp, then in-place pe merge (E1 -> E1', DVE) and the
                    # banded E2 = E1' * band (Pool). Strip tt covers s in
                    # [128tt-32, 128tt+160); local l in [lo, hi) clipped.
                    e1 = epool.tile(
                        [128, S], bf16, name=f"e1{self.gtag}_{tt}", tag="e1"
                    )
                    nc.scalar.activation(
                        e1[:], self.sc[tt][:], Exp, scale=EXP_SCALE
                    )
                    self.E1s[tt] = e1
                    lo = 32 if tt == 0 else 0
                    hi = 160 if tt == 3 else 192
                    reg = e1[:, 128 * tt - 32 + lo : 128 * tt - 32 + hi]
                    nc.vector.tensor_tensor(reg, reg, estrip_sb[:, lo:hi], mult)
                    st = stpool.tile(
                        [128, 192], bf16, name=f"e2{self.gtag}_{tt}", tag="e2"
                    )
                    nc.gpsimd.tensor_tensor(
                        st[:, lo:hi], reg, band_sb[:, lo:hi], mult
                    )
                    self.E2s[tt] = st

                def run_all(self):
                    for tt in range(4):
                        self.mm(tt)
                        self.expstrip(tt)

            pending = []  # weaveable big-MM thunks (qkv / outproj chunks)

            def head_back(h, vslot, ET, E1s, E2s, nxt=None):
                """transposed AV + wide normalization + blend for head h."""
                et, hl = h // 2, h % 2
                # ---- transposed AV: per s-chunk st, out [128s, 65] =
                # E1'^T @ V~ (V~ = [V | ones] moving, N=65). Col 64 is the
                # per-s denominator -> wide per-partition reciprocal. ----
                PT1 = popool.tile([128, 4, 128], f32, tag="po")
                PT2 = popool.tile([128, 4, 128], f32, tag="po")
                vaug = vt_all[:, vslot]
                for st in range(4):
                    # weave the next head-pair's scores (row-group-paired
                    # matmuls stream concurrently) between the small AVT
                    # clusters to keep the PE array dense
                    if nxt is not None:
                        fa, fb = nxt
                        fa.mm(st)
                        fb.mm(st)
                        fa.expstrip(st)
                        fb.expstrip(st)
                    for tt in range(4):
                        nc.tensor.matmul(
                            PT1[:, st, 0:65],
                            E1s[tt][:, 128 * st : 128 * (st + 1)],
                            vaug[:, tt, h, :],
                            start=(tt == 0),
                            stop=(tt == 3),
                            skip_group_check=True,
                        )
                    # band: main strip tt=st covers the whole chunk; strip
                    # tt=st-1 covers s-subrange [0:32), tt=st+1 [96:128).
                    nc.tensor.matmul(
                        PT2[:, st, 0:65],
                        E2s[st][:, 32:160],
                        vaug[:, st, h, :],
                        start=True,
                        stop=False,
                        skip_group_check=True,
                    )
                    if st > 0:
                        nc.tensor.matmul(
                            PT2[0:32, st, 0:65],
                            E2s[st - 1][:, 160:192],
                            vaug[:, st - 1, h, :],
                            start=False,
                            stop=(st == 3),
                            skip_group_check=True,
                        )
                    if st < 3:
                        nc.tensor.matmul(
                            PT2[96:128, st, 0:65],
                            E2s[st + 1][:, 0:32],
                            vaug[:, st + 1, h, :],
                            start=False,
                            stop=True,
                            skip_group_check=True,
                            tile_position=(0, 96),
                        )

                # ---- normalization, all wide: [128,4] reciprocals,
                # free-dim-broadcast multiplies, blend into the ET tile ----
                rT1 = rpool.tile([128, 4], f32, tag="rt1")
                nc.vector.reciprocal(rT1[:], PT1[:, :, 64:65])
                rT2 = rpool.tile([128, 4], f32, tag="rt2")
                nc.vector.reciprocal(rT2[:], PT2[:, :, 64:65])
                tT1 = npool.tile([128, 4, 64], bf16, tag="t1")
                nc.vector.tensor_tensor(
                    tT1[:], PT1[:, :, 0:64],
                    rT1[:, :, None].broadcast_to((128, 4, 64)), mult,
                )
                tT2 = npool.tile([128, 4, 64], bf16, tag="t2")
                nc.vector.tensor_tensor(
                    tT2[:], PT2[:, :, 0:64],
                    rT2[:, :, None].broadcast_to((128, 4, 64)), mult,
                )
                nc.gpsimd.tensor_tensor(ET[et][:, :, hl, :], tT1[:], tT2[:], add)

            # ---- software-pipelined head stream: emit scores/exp/strips
            # for head g+1 BEFORE the AV/normalize of head g, so the PE
            # chews AV(g) while ACT/DVE/Pool produce head g+1's strips ----
            NG = BPC * H
            bctx = {0: (QP, KP, vslot)}  # per-batch (QP, KP, vslot)
            ET_all = {}
            OT_all = {}
            fronts = {}
            fronts[0] = Front(0, QP, KP, "g0")
            fronts[1] = Front(1, QP, KP, "g1")
            for tt in range(4):
                for f in (fronts[0], fronts[1]):
                    f.mm(tt)
                for f in (fronts[0], fronts[1]):
                    f.expstrip(tt)
            for g in range(NG):
                b, h = g // H, g % H
                if h == 0:
                    ET_all[b] = [
                        etpool.tile(
                            [128, 4, 2, 64], bf16, name=f"et{c}_{b}",
                            tag=f"et{c}",
                        )
                        for c in range(2)
                    ]
                    OT_all[b] = [None, None]
                    if b + 2 < BPC:
                        # prefetch x two batches ahead: the sync DMA queue
                        # also carries the OT transposes and out stores, so
                        # a one-batch prefetch arrives ~3.5us late
                        xts[b + 2] = load_xt(b + 2)
                nxt = None
                if h % 2 == 0 and g + 2 < NG:
                    # prepare the NEXT et-pair's fronts, woven into this
                    # back as row-group-paired score matmuls
                    QPf, KPf, _ = bctx[(g + 2) // H]
                    fa = Front((g + 2) % H, QPf, KPf, f"g{g + 2}")
                    fb = Front((g + 3) % H, QPf, KPf, f"g{g + 3}")
                    fronts[g + 2], fronts[g + 3] = fa, fb
                    nxt = (fa, fb)
                fr = fronts.pop(g)
                _, _, vs = bctx[b]
                head_back(h, vs, ET_all[b], fr.E1s, fr.E2s, nxt=nxt)
                if h % 2 == 1:
                    # assemble OT[et] = ET[et].T via the DMA XBAR
                    et = h // 2
                    ot = otpool.tile(
                        [128, S], bf16, name=f"ot{et}_{b}", tag=f"ot{et}"
                    )
                    for st in range(4):
                        nc.sync.dma_start(
                            ot[:, 128 * st : 128 * (st + 1)],
                            ET_all[b][et][:, st, :, :],
                            transpose=True,
                        )
                    OT_all[b][et] = ot
                if h == 0 and b > 0:
                    # deferred out-proj of the previous batch
                    do_outproj(OT_all[b - 1], b - 1)
                if h == 1 and b + 1 < BPC:
                    # next batch's projections early
                    bctx[b + 1] = qkv_proj(*xts.pop(b + 1), b + 1)
            do_outproj(OT_all[BPC - 1], BPC - 1)

    nc.compile()
    return nc


_CACHE = {}
LAST_RESULTS = None


def prep_in_maps(inputs, Wq, bq, Wk, bk, Wv, bv, gamma, theta, Wout, bout):
    import ml_dtypes

    bfloat16 = ml_dtypes.bfloat16

    x = np.asarray(inputs, np.float32)
    Wq = np.asarray(Wq, np.float32)
    bq = np.asarray(bq, np.float32)
    Wk = np.asarray(Wk, np.float32)
    bk = np.asarray(bk, np.float32)
    Wv = np.asarray(Wv, np.float32)
    bv = np.asarray(bv, np.float32)
    Wout = np.asarray(Wout, np.float32)
    bout = np.asarray(bout, np.float32)
    gamma = float(np.asarray(gamma))
    theta = float(np.asarray(theta))

    # host-side prep. W{q,k} scaled by 32 for fp8 range; the projection
    # outputs are then 32x, scores 1024x -> compensated in EXP_SCALE
    # (with the softmax 1/sqrt(E)).
    WSC = 32.0
    fp8 = ml_dtypes.float8_e4m3
    wq_8 = (WSC * Wq).astype(fp8)
    wk_8 = (WSC * Wk).astype(fp8)
    wv_b = Wv.astype(bfloat16)
    qkb = (WSC * np.stack(
        [bq[:128], bq[128:], bk[:128], bk[128:]], axis=1
    )).astype(np.float32)  # [128, 4]
    bout_p = (bout + bv @ Wout).astype(np.float32).reshape(1, F)
    wout_h = (0.5 * Wout).astype(bfloat16)
    # strip coords: l = s - (128tt - 32); delta = t - s = p - l + 32.
    # estrip = exp(pe(delta)) (== 1 in bf16 beyond |delta|<=2);
    # bandmask = 1 where |delta| <= HALF_WIN else 0.
    p_i = np.arange(128)[:, None]
    l_i = np.arange(192)[None, :]
    delta = (p_i - l_i + 32).astype(np.float32)
    pe_val = np.exp(-np.abs(gamma * delta * delta - theta)).astype(np.float32)
    band = (np.abs(delta) <= HALF_WIN).astype(np.float32)
    estrip = np.exp(pe_val).astype(bfloat16)
    bandmask = band.astype(bfloat16)

    shared = {
        "wq": np.ascontiguousarray(wq_8),
        "wk": np.ascontiguousarray(wk_8),
        "wv": np.ascontiguousarray(wv_b),
        "wout": np.ascontiguousarray(wout_h),
        "qkbias": np.ascontiguousarray(qkb),
        "boutr": bout_p,
        "estrip": np.ascontiguousarray(estrip),
        "bandmask": np.ascontiguousarray(bandmask),
    }
    in_maps = []
    for c in range(NCORES):
        xc = x[c * BPC : (c + 1) * BPC].reshape(TOK, F)
        m = dict(shared)
        xct = xc.T
        m["xT"] = np.ascontiguousarray(xct.astype(fp8))
        m["xTv"] = np.ascontiguousarray(xct.astype(bfloat16))
        in_maps.append(m)
    return in_maps


def get_nc():
    if "nc" not in _CACHE:
        _CACHE["nc"] = _build()
    return _CACHE["nc"]


def kernel(inputs, Wq, bq, Wk, bk, Wv, bv, gamma, theta, Wout, bout):
    global LAST_RESULTS
    from concourse.bass_utils import run_bass_kernel_spmd

    in_maps = prep_in_maps(
        inputs, Wq, bq, Wk, bk, Wv, bv, gamma, theta, Wout, bout
    )
    nc = get_nc()
    res = run_bass_kernel_spmd(nc, in_maps, core_ids=list(range(NCORES)))
    LAST_RESULTS = res
    out = np.concatenate(
        [res.results[c]["out"].reshape(BPC, S, F) for c in range(NCORES)], axis=0
    )
    return out
